# revision 1
# baseline (speedup 1.0000x reference)
"""Trainium2 kernel for nn_CODABlocks2D: CODA transformer block over 2D fields.

Strategy (sharding_hint): attention contracts over T within each batch
element -> shard the 64 (batch, head) attention pairs across the 8 cores
(8 pairs per core).  The attention core (QK^T, softmax, AV) runs on-device
via Bass/Tile; the FNO/FFT/normalizer stages run per-token on host (numpy,
fp32-equivalent math identical to the reference).
"""

import math
import sys

import numpy as np

sys.path.insert(0, "/opt/trn_rl_repo")

EPS = 1e-5
N_HEADS = 32
B, T, H, W = 2, 32, 128, 128

LAST_EXEC_NS = None


# ---------------------------------------------------------------------------
# Host math (numpy ports of the jax reference; fp32 in/out)
# ---------------------------------------------------------------------------

def _inorm(x, g, b):
    m = x.mean(axis=(-2, -1), keepdims=True, dtype=np.float64)
    v = ((x - m) ** 2).mean(axis=(-2, -1), keepdims=True, dtype=np.float64)
    out = (x - m) / np.sqrt(v + EPS) * g + b
    return out.astype(np.float32)


def _resample_half(x):
    # Fourier resample (128,128) -> (64,64), norm='forward'
    xf = np.fft.rfft2(x, norm="forward")
    kh, kw = 32, 33
    of = np.zeros(x.shape[:-2] + (64, 33), dtype=np.complex128)
    of[..., :kh, :kw] = xf[..., :kh, :kw]
    of[..., -kh:, :kw] = xf[..., -kh:, :kw]
    return np.fft.irfft2(of, s=(64, 64), norm="forward").astype(np.float32)


def _spec_conv(x, w, out_hw):
    m1, m2 = w.shape[3], w.shape[4]
    Ho, Wo = out_hw
    wc = (w[..., 0] + 1j * w[..., 1]).astype(np.complex128)  # [2, in, out, m1, m2]
    xf = np.fft.rfft2(x, norm="forward")  # [B, Cin, H, W//2+1]
    top = np.einsum("bimn,iomn->bomn", xf[:, :, :m1, :m2], wc[0])
    bot = np.einsum("bimn,iomn->bomn", xf[:, :, -m1:, :m2], wc[1])
    of = np.zeros((x.shape[0], w.shape[2], Ho, Wo // 2 + 1), dtype=np.complex128)
    of[:, :, :m1, :m2] = top
    of[:, :, -m1:, :m2] = bot
    return np.fft.irfft2(of, s=(Ho, Wo), norm="forward").astype(np.float32)


try:
    from scipy.special import erf as _erf
except Exception:  # pragma: no cover
    _erf = np.vectorize(math.erf, otypes=[np.float64])


def _gelu(x):
    x64 = x.astype(np.float64)
    return (0.5 * x64 * (1.0 + _erf(x64 / math.sqrt(2.0)))).astype(np.float32)


def _fno_layer(x, w, ws, bs, out_hw, norm_gb=None, act=False):
    skip = np.einsum("bchw,oc->bohw", x, ws) + bs[None, :, None, None]
    if out_hw != x.shape[-2:]:
        skip = _resample_half(skip)
    fno = _spec_conv(x, w, out_hw)
    if norm_gb is not None:
        fno = _inorm(fno, norm_gb[0], norm_gb[1])
    y = (fno + skip).astype(np.float32)
    if act:
        y = _gelu(y)
    return y


def _to_seq(z):
    h, w = z.shape[-2:]
    z = z.reshape(B, T, N_HEADS, 1, h, w).transpose(0, 2, 1, 3, 4, 5)
    return np.ascontiguousarray(z.reshape(B, N_HEADS, T, h * w))


# ---------------------------------------------------------------------------
# Device kernel: attention core for 8 (b,h) pairs per core
#   scores = qs @ ks^T / 64 ; softmax ; out = attn @ vs
# ---------------------------------------------------------------------------

_NC = None


def _build_nc():
    import concourse.bacc as bacc
    import concourse.mybir as mybir
    from concourse.tile import TileContext

    f32 = mybir.dt.float32
    X = mybir.AxisListType.X
    Exp = mybir.ActivationFunctionType.Exp

    # Bacc (not Bass): its pipeline runs generate_event_semaphores, which
    # splits multi-sem sync waits to satisfy the TRN2 per-instruction limit
    nc = bacc.Bacc(None, target_bir_lowering=False)
    qT = nc.dram_tensor("qT8", [8, 4096, 32], f32, kind="ExternalInput")
    kT = nc.dram_tensor("kT8", [8, 4096, 32], f32, kind="ExternalInput")
    v = nc.dram_tensor("v8", [8, 32, 16384], f32, kind="ExternalInput")
    o = nc.dram_tensor("o8", [8, 32, 16384], f32, kind="ExternalOutput")

    with TileContext(nc) as tc:
        with tc.tile_pool(name="io", bufs=2) as io_pool, \
             tc.tile_pool(name="vio", bufs=1) as vio_pool, \
             tc.tile_pool(name="sm", bufs=2) as sm_pool, \
             tc.tile_pool(name="ps", bufs=2, space="PSUM") as ps_pool, \
             tc.tile_pool(name="pso", bufs=4, space="PSUM") as pso_pool:
            for p in range(8):
                qraw = io_pool.tile([128, 1024], f32, tag="qraw")
                kraw = io_pool.tile([128, 1024], f32, tag="kraw")
                nc.sync.dma_start(
                    qraw.rearrange("q (c t) -> q c t", c=32),
                    qT[p].rearrange("(c q) t -> q c t", q=128))
                nc.sync.dma_start(
                    kraw.rearrange("q (c t) -> q c t", c=32),
                    kT[p].rearrange("(c q) t -> q c t", q=128))
                # single DVE copy so downstream matmuls wait on one
                # engine sem instead of the DMA's many HW-queue sems
                # (walrus: "Too many sync wait commands" on Matmult)
                qt = io_pool.tile([128, 1024], f32, tag="qt")
                kt = io_pool.tile([128, 1024], f32, tag="kt")
                nc.vector.tensor_copy(qt, qraw)
                nc.vector.tensor_copy(kt, kraw)
                ps_sc = ps_pool.tile([32, 32], f32, tag="ps_sc")
                for c in range(32):
                    nc.tensor.matmul(ps_sc, qt[:, 32 * c:32 * c + 32],
                                     kt[:, 32 * c:32 * c + 32],
                                     start=(c == 0), stop=(c == 31))
                sc = sm_pool.tile([32, 32], f32, tag="sc")
                nc.scalar.mul(sc, ps_sc, 1.0 / 64.0)
                mx = sm_pool.tile([32, 1], f32, tag="mx")
                nc.vector.reduce_max(mx, sc, axis=X)
                nmx = sm_pool.tile([32, 1], f32, tag="nmx")
                nc.scalar.mul(nmx, mx, -1.0)
                ex = sm_pool.tile([32, 32], f32, tag="ex")
                nc.scalar.activation(ex, sc, Exp, bias=nmx[:, 0:1])
                smv = sm_pool.tile([32, 1], f32, tag="smv")
                nc.vector.reduce_sum(smv, ex, axis=X)
                rc = sm_pool.tile([32, 1], f32, tag="rc")
                nc.vector.reciprocal(rc, smv)
                at = sm_pool.tile([32, 32], f32, tag="at")
                nc.vector.tensor_scalar_mul(at, ex, rc[:, 0:1])
                atT = sm_pool.tile([32, 32], f32, tag="atT")
                nc.vector.transpose(atT, at)
                for half in range(2):
                    hof = 8192 * half
                    vraw = vio_pool.tile([32, 8192], f32, tag="vraw")
                    nc.sync.dma_start(vraw, v[p, :, hof:hof + 8192])
                    vall = vio_pool.tile([32, 8192], f32, tag="vall")
                    nc.vector.tensor_copy(vall, vraw)
                    oall = vio_pool.tile([32, 8192], f32, tag="oall")
                    for j in range(16):
                        po = pso_pool.tile([32, 512], f32, tag="po")
                        nc.tensor.matmul(po, atT,
                                         vall[:, 512 * j:512 * j + 512],
                                         start=True, stop=True)
                        nc.vector.tensor_copy(
                            oall[:, 512 * j:512 * j + 512], po)
                    nc.sync.dma_start(o[p, :, hof:hof + 8192], oall)
    nc.compile()
    return nc


def _attention_device(qs, ks, vs):
    """qs/ks: [B, nH, T, 4096]; vs: [B, nH, T, 16384] -> out like vs."""
    global _NC, LAST_EXEC_NS
    import time

    import concourse.bass_utils as bass_utils

    if _NC is None:
        _NC = _build_nc()

    qp = qs.reshape(64, T, 4096)
    kp = ks.reshape(64, T, 4096)
    vp = np.ascontiguousarray(vs.reshape(64, T, 16384))
    in_maps = []
    for c in range(8):
        in_maps.append({
            "qT8": np.ascontiguousarray(
                qp[8 * c:8 * c + 8].transpose(0, 2, 1)),
            "kT8": np.ascontiguousarray(
                kp[8 * c:8 * c + 8].transpose(0, 2, 1)),
            "v8": vp[8 * c:8 * c + 8],
        })
    t0 = time.time()
    res = bass_utils.run_bass_kernel_spmd(_NC, in_maps, core_ids=list(range(8)))
    t1 = time.time()
    LAST_EXEC_NS = (res.exec_time_ns if res.exec_time_ns
                    else int((t1 - t0) * 1e9))
    out = np.concatenate([np.asarray(r["o8"]) for r in res.results], axis=0)
    return out.reshape(B, N_HEADS, T, H * W)


# ---------------------------------------------------------------------------
# Full forward
# ---------------------------------------------------------------------------

def kernel(x, wK, wKs, bKs, wQ, wQs, bQs, wV, wVs, bVs, wP, wPs, bPs,
           wM0, wM0s, bM0s, wM1, wM1s, bM1s, norm_g, norm_b):
    x = np.asarray(x, dtype=np.float32)
    args = {k: np.asarray(val, dtype=np.float32) for k, val in [
        ("wK", wK), ("wKs", wKs), ("bKs", bKs), ("wQ", wQ), ("wQs", wQs),
        ("bQs", bQs), ("wV", wV), ("wVs", wVs), ("bVs", bVs), ("wP", wP),
        ("wPs", wPs), ("bPs", bPs), ("wM0", wM0), ("wM0s", wM0s),
        ("bM0s", bM0s), ("wM1", wM1), ("wM1s", wM1s), ("bM1s", bM1s),
        ("norm_g", norm_g), ("norm_b", norm_b)]}
    g = args["norm_g"]
    b = args["norm_b"]

    xa = x.reshape(B * T, 1, H, W)
    xa_n = _inorm(xa, g[0], b[0])
    k_img = _fno_layer(xa_n, args["wK"], args["wKs"], args["bKs"], (64, 64))
    q_img = _fno_layer(xa_n, args["wQ"], args["wQs"], args["bQs"], (64, 64))
    v_img = _fno_layer(xa_n, args["wV"], args["wVs"], args["bVs"], (128, 128))

    qs, ks, vs = _to_seq(q_img), _to_seq(k_img), _to_seq(v_img)
    out = _attention_device(qs, ks, vs)

    out = out.reshape(B, N_HEADS, T, 1, H, W).transpose(0, 2, 1, 3, 4, 5)
    out = np.ascontiguousarray(out.reshape(B * T, N_HEADS, H, W))

    projd = _fno_layer(out, args["wP"], args["wPs"], args["bPs"], (128, 128))
    attention = _inorm(projd + xa, g[1], b[1])
    an = _inorm(attention, g[2], b[2])
    m = _fno_layer(an, args["wM0"], args["wM0s"], args["bM0s"], (128, 128),
                   (g[3], b[3]), act=True)
    m = _fno_layer(m, args["wM1"], args["wM1s"], args["bM1s"], (128, 128),
                   (g[4], b[4]), act=False)
    output = _inorm(m, g[5], b[5]) + attention
    return np.ascontiguousarray(output.reshape(B, T, H, W).astype(np.float32))



# revision 2
# speedup vs baseline: 38.5977x; 38.5977x over previous
"""Trainium2 kernel for nn_CODABlocks2D: full CODA block on-device.

All stages (instance norms, FNO spectral convs, attention, projection,
mixer) run on the NeuronCores in the Fourier domain; FFTs are PE matmuls
against DFT matrices. Sharding: core c handles batch c//4 and the 8-token
output shard c%4 (tokens pre-rotated per core so the shard is always
tokens 0..8; the pipeline is permutation-equivariant over tokens).
Per-core transfer: ~3.5 MB in / 0.5 MB out.
"""
import sys

sys.path.insert(0, "/opt/trn_rl_repo")

import numpy as np

EPS = 1e-5
NH, T, NS = 32, 32, 8
B, HH, WW = 2, 128, 128

# mode windows (source row indices in the 128-row spectrum)
RP = np.r_[0:32, 96:128]
R16_SRC = np.r_[0:16, 112:128]

LAST_EXEC_NS = None
_NC = None
_CONSTS = None

def dft_constants():
    h = np.arange(128)
    c = {}
    ang = 2 * np.pi * np.outer(h, h) / 128.0
    c["CA"] = (np.cos(ang) / 16384.0).astype(np.float32)       # [h, r] passA
    c["SA"] = (-np.sin(ang) / 16384.0).astype(np.float32)
    angw = 2 * np.pi * np.outer(h, np.arange(65)) / 128.0
    c["LC"] = np.cos(angw).astype(np.float32)                  # [w, c] passB
    c["LSP"] = np.sin(angw).astype(np.float32)
    c["LSN"] = (-np.sin(angw)).astype(np.float32)
    c["CAR"] = np.cos(ang).astype(np.float32)                  # [r, h] irfft1
    c["SAR"] = np.sin(ang).astype(np.float32)
    g = np.full(65, 2.0); g[0] = 1.0; g[64] = 1.0
    angc = 2 * np.pi * np.outer(np.arange(65), h) / 128.0
    c["GC"] = (g[:, None] * np.cos(angc)).astype(np.float32)   # [c, w] irfft2
    c["GS"] = (-g[:, None] * np.sin(angc)).astype(np.float32)
    # M-layer: windowed rfft2 (rows RP, cols 0..31) and irfft from window
    c["CAM"] = (np.cos(2 * np.pi * np.outer(h, RP) / 128.0) / 16384.0
                ).astype(np.float32)                           # [h, 64]
    c["SAM"] = (-np.sin(2 * np.pi * np.outer(h, RP) / 128.0) / 16384.0
                ).astype(np.float32)
    angm = 2 * np.pi * np.outer(h, np.arange(32)) / 128.0
    c["LCM"] = np.cos(angm).astype(np.float32)                 # [w, 32]
    c["LSPM"] = np.sin(angm).astype(np.float32)
    c["LSNM"] = (-np.sin(angm)).astype(np.float32)
    gm = np.full(32, 2.0); gm[0] = 1.0
    angc2 = 2 * np.pi * np.outer(np.arange(32), h) / 128.0
    c["GCM"] = (gm[:, None] * np.cos(angc2)).astype(np.float32)   # [32 c, w]
    c["GSM"] = (gm[:, None] * np.sin(angc2)).astype(np.float32)
    c["GSMN"] = (-gm[:, None] * np.sin(angc2)).astype(np.float32)
    c["CARM"] = np.cos(2 * np.pi * np.outer(RP, h) / 128.0).astype(np.float32)
    c["SARM"] = np.sin(2 * np.pi * np.outer(RP, h) / 128.0).astype(np.float32)
    c["IDEN"] = np.eye(128, dtype=np.float32)
    return c


def weight_constants(wts):
    (wK, wKs, wQ, wQs, wV, wVs, wP, wPs, wM0, wM0s, wM1, wM1s, g, b) = wts
    cc = {}
    WQc = wQ[..., 0] + 1j * wQ[..., 1]
    WKc = wK[..., 0] + 1j * wK[..., 1]
    WVc = wV[..., 0] + 1j * wV[..., 1]
    WPc = wP[..., 0] + 1j * wP[..., 1]
    WM0c = wM0[..., 0] + 1j * wM0[..., 1]
    WM1c = wM1[..., 0] + 1j * wM1[..., 1]

    # --- scores: per-c column tiles over ALL 128 src rows -------------
    # tile c (0..32): modes = (c, src row r 0..127); weights zero outside
    # the 64-window rows {0..31, 96..127}; 16-block at (c<16, r in
    # {0..15, 112..127}); wRvec on 64-window minus 16-block.
    NT = 33
    W2RE = np.zeros((NT, 128, NH), np.float32)
    W2IMN = np.zeros((NT, 128, NH), np.float32)
    wRv = np.zeros((NT, 128, 1), np.float32)
    wc = np.full(33, 2.0); wc[0] = 1.0; wc[32] = 1.0
    scl = 4096.0 / 64.0
    Wq_rows = {h: np.concatenate([WQc[0, 0, h], WQc[1, 0, h]], 0)
               for h in range(NH)}
    Wk_rows = {h: np.concatenate([WKc[0, 0, h], WKc[1, 0, h]], 0)
               for h in range(NH)}
    for cix in range(33):
        for r in range(128):
            in64 = r < 32 or r >= 96
            if not in64:
                continue
            in16 = cix < 16 and (r < 16 or r >= 112)
            if in16:
                ri = r if r < 16 else r - 96  # 112..127 -> 16..31
                for h in range(NH):
                    wq = Wq_rows[h][ri, cix] + wQs[h, 0]
                    wk = Wk_rows[h][ri, cix] + wKs[h, 0]
                    w2 = wq * np.conj(wk) * wc[cix] * scl
                    W2RE[cix, r, h] = w2.real
                    W2IMN[cix, r, h] = -w2.imag
            else:
                wRv[cix, r, 0] = wc[cix] * scl
    cc["W2RE"] = W2RE
    cc["W2IMN"] = W2IMN
    cc["WRV"] = wRv
    cc["SQSK"] = (wQs[:, 0] * wKs[:, 0]).reshape(1, NH).astype(np.float32)

    # --- proj stage per-column weight planes --------------------------
    # WVRE/WVIM [128 r, 32 c, NH]: sv + Wv(embedded rows R16_SRC, c<16)
    # WP2RE/WP2IM [128 r, 32 c, NH]: (WP + wps) on RP rows, else 0
    sv = wVs[:, 0]
    wps = wPs[0]
    WVRE = np.zeros((128, 32, NH), np.float32)
    WVIM = np.zeros((128, 32, NH), np.float32)
    WVRE[:] = sv[None, None, :]
    Wvfull = np.concatenate([WVc[0, 0], WVc[1, 0]], 1)  # [NH? no:
    # WVc[0,0]: [NH,16,16] -> rows block; build [32 rows,16 c, NH]
    Wv_rows = np.concatenate([WVc[0, 0].transpose(1, 2, 0),
                              WVc[1, 0].transpose(1, 2, 0)], 0)  # [32,16,NH]
    WVRE[R16_SRC, :16, :] += Wv_rows.real
    WVIM[R16_SRC, :16, :] += Wv_rows.imag
    WP2RE = np.zeros((128, 32, NH), np.float32)
    WP2IM = np.zeros((128, 32, NH), np.float32)
    Wp_rows = np.concatenate([WPc[0, :, 0].transpose(1, 2, 0),
                              WPc[1, :, 0].transpose(1, 2, 0)], 0)  # [64,32,NH]
    WP2RE[RP, :, :] = Wp_rows.real + wps[None, None, :]
    WP2IM[RP, :, :] = Wp_rows.imag
    cc["WVRE"], cc["WVIM"] = WVRE, WVIM
    cc["WP2RE"], cc["WP2IM"] = WP2RE, WP2IM
    cc["WPSSV"] = (wps * sv).reshape(NH, 1).astype(np.float32)

    # --- M layers -----------------------------------------------------
    for nm, Wc in [("WM0", WM0c), ("WM1", WM1c)]:
        Wm = np.concatenate([Wc[0, 0, 0], Wc[1, 0, 0]], 0)  # [64 rP, 32 c]
        cc[nm + "RE"] = Wm.real.T.astype(np.float32).copy()  # [32 c, 64 r]
        cc[nm + "IM"] = Wm.imag.T.astype(np.float32).copy()
    cc["SCAL"] = np.array([g[0], b[0], g[1], b[1], g[2], b[2], g[3], b[3],
                           g[4], b[4], g[5], b[5], wM0s[0, 0], wM1s[0, 0]],
                          np.float32)
    return cc



class StopStage(Exception):
    pass


def build_nc(debug=False, stage=99):
    import concourse.bacc as bacc
    import concourse.mybir as mybir
    from concourse.tile import TileContext

    f32 = mybir.dt.float32
    X = mybir.AxisListType.X
    Alu = mybir.AluOpType
    Act = mybir.ActivationFunctionType

    nc = bacc.Bacc(None, target_bir_lowering=False)

    def dt(name, shape, kind="ExternalInput"):
        return nc.dram_tensor(name, shape, f32, kind=kind)

    x32 = dt("x32", [32, 16384])
    CA, SA = dt("CA", [128, 128]), dt("SA", [128, 128])
    LC, LSP, LSN = dt("LC", [128, 65]), dt("LSP", [128, 65]), dt("LSN", [128, 65])
    CAR, SAR = dt("CAR", [128, 128]), dt("SAR", [128, 128])
    GC, GS = dt("GC", [65, 128]), dt("GS", [65, 128])
    CAM, SAM = dt("CAM", [128, 64]), dt("SAM", [128, 64])
    LCM, LSPM, LSNM = (dt("LCM", [128, 32]), dt("LSPM", [128, 32]),
                       dt("LSNM", [128, 32]))
    GCM, GSM, GSMN = (dt("GCM", [32, 128]), dt("GSM", [32, 128]),
                      dt("GSMN", [32, 128]))
    CARM, SARM = dt("CARM", [64, 128]), dt("SARM", [64, 128])
    IDEN = dt("IDEN", [128, 128])
    IDENS = dt("IDENS", [64, 32])
    W2RE, W2IMN = dt("W2RE", [33, 128, 32]), dt("W2IMN", [33, 128, 32])
    WRV = dt("WRV", [33, 128, 1])
    SQSK = dt("SQSK", [1, 32])
    WVRE, WVIM = dt("WVRE", [128, 1024]), dt("WVIM", [128, 1024])
    WP2RE, WP2IM = dt("WP2RE", [128, 1024]), dt("WP2IM", [128, 1024])
    WPSSV = dt("WPSSV", [32, 1])
    WM0RE, WM0IM = dt("WM0RE", [32, 64]), dt("WM0IM", [32, 64])
    WM1RE, WM1IM = dt("WM1RE", [32, 64]), dt("WM1IM", [32, 64])
    SC32 = dt("SC32", [32, 16])
    SC128 = dt("SC128", [128, 16])
    ONES128 = dt("ONES128", [128, 1])
    ONES1 = dt("ONES1", [1, 128])
    OUT = dt("OUT", [128, 1024], kind="ExternalOutput")
    ZSCR_RE = dt("ZSCR_RE", [65, 4096], kind="Internal")
    ZSCR_IM = dt("ZSCR_IM", [65, 4096], kind="Internal")
    ASCR = dt("ASCR", [32, 1], kind="Internal")
    CSCR = dt("CSCR", [1, 256], kind="Internal")
    TSCR1 = dt("TSCR1", [128, 32], kind="Internal")
    TSCR2 = dt("TSCR2", [128, 32], kind="Internal")
    if debug:
        DXF = dt("DXF", [64, 8320], kind="ExternalOutput")
        DT1 = dt("DT1", [128, 4096], kind="ExternalOutput")
        DT2 = dt("DT2", [128, 4096], kind="ExternalOutput")
        DT3 = dt("DT3", [128, 4096], kind="ExternalOutput")
        DT4 = dt("DT4", [65, 4096], kind="ExternalOutput")
        DT5 = dt("DT5", [64, 8320], kind="ExternalOutput")
        DCT = dt("DCT", [64, 8], kind="ExternalOutput")
        DATT = dt("DATT", [64, 256], kind="ExternalOutput")
        DT6 = dt("DT6", [64, 4], kind="ExternalOutput")
        DSC = dt("DSC", [32, 256], kind="ExternalOutput")
        DAT = dt("DAT", [32, 256], kind="ExternalOutput")
        DZPR = dt("DZPR", [128, 520], kind="ExternalOutput")
        DZPI = dt("DZPI", [128, 520], kind="ExternalOutput")
        DPJ = dt("DPJ", [128, 1024], kind="ExternalOutput")
        DAN = dt("DAN", [128, 1024], kind="ExternalOutput")

    with TileContext(nc) as tc:
        with tc.tile_pool(name="const", bufs=1) as cp, \
             tc.tile_pool(name="io", bufs=1) as io, \
             tc.tile_pool(name="work", bufs=1) as wk, \
             tc.tile_pool(name="work2", bufs=2) as w2p, \
             tc.tile_pool(name="pa", bufs=2, space="PSUM") as pa, \
             tc.tile_pool(name="pb", bufs=2, space="PSUM") as pb, \
             tc.tile_pool(name="pc", bufs=2, space="PSUM") as pc, \
             tc.tile_pool(name="pacc", bufs=1, space="PSUM") as pacc:
            def load(t, shape, name):
                tl = cp.tile(shape, f32, tag=name, name=name)
                nc.sync.dma_start(tl, t[(slice(None),) * len(shape)])
                return tl

            ca, sa = load(CA, [128, 128], "ca"), load(SA, [128, 128], "sa")
            lc = load(LC, [128, 65], "lc")
            lsp = load(LSP, [128, 65], "lsp")
            lsn = load(LSN, [128, 65], "lsn")
            car, sar = load(CAR, [128, 128], "car"), load(SAR, [128, 128], "sar")
            gc_, gs_ = load(GC, [65, 128], "gc"), load(GS, [65, 128], "gs")
            cam, sam = load(CAM, [128, 64], "cam"), load(SAM, [128, 64], "sam")
            lcm = load(LCM, [128, 32], "lcm")
            lspm = load(LSPM, [128, 32], "lspm")
            lsnm = load(LSNM, [128, 32], "lsnm")
            gcm = load(GCM, [32, 128], "gcm")
            gsm = load(GSM, [32, 128], "gsm")
            gsmn = load(GSMN, [32, 128], "gsmn")
            carm = load(CARM, [64, 128], "carm")
            sarm = load(SARM, [64, 128], "sarm")
            iden = load(IDEN, [128, 128], "iden")
            idens = load(IDENS, [64, 32], "idens")
            w2re = cp.tile([128, 33 * 32], f32, tag="w2re", name="w2re")
            w2imn = cp.tile([128, 33 * 32], f32, tag="w2imn", name="w2imn")
            wrv = cp.tile([128, 33], f32, tag="wrv", name="wrv")
            nc.sync.dma_start(w2re.rearrange("p (k h) -> p k h", k=33),
                              W2RE[:, :, :].rearrange("k p h -> p k h"))
            nc.sync.dma_start(w2imn.rearrange("p (k h) -> p k h", k=33),
                              W2IMN[:, :, :].rearrange("k p h -> p k h"))
            nc.sync.dma_start(wrv.rearrange("p (k o) -> p k o", k=33),
                              WRV[:, :, :].rearrange("k p o -> p k o"))
            sqsk = load(SQSK, [1, 32], "sqsk")
            wvre = load(WVRE, [128, 1024], "wvre")
            wvim = load(WVIM, [128, 1024], "wvim")
            wp2re = load(WP2RE, [128, 1024], "wp2re")
            wp2im = load(WP2IM, [128, 1024], "wp2im")
            wpssv = load(WPSSV, [32, 1], "wpssv")
            wm0re = load(WM0RE, [32, 64], "wm0re")
            wm0im = load(WM0IM, [32, 64], "wm0im")
            wm1re = load(WM1RE, [32, 64], "wm1re")
            wm1im = load(WM1IM, [32, 64], "wm1im")
            sc32 = load(SC32, [32, 16], "sc32")
            sc128 = load(SC128, [128, 16], "sc128")
            ones128 = load(ONES128, [128, 1], "ones128")
            ones1 = load(ONES1, [1, 128], "ones1")

            TT = nc.vector.tensor_tensor
            TS = nc.vector.tensor_scalar_mul
            CP_ = nc.vector.tensor_copy

            def big(tag, shape):
                return io.tile(shape, f32, tag=tag, name=tag + "_t")

            def small(tag, shape, pool=None):
                return (pool or wk).tile(shape, f32, tag=tag, name=tag + "_t")

            # ---------------- load + stats (from [h,(t,w)] layout) --------
            xh = big("bigC", [128, 4096])
            nc.sync.dma_start(xh.rearrange("h (t w) -> h t w", t=32),
                              x32[:, :].rearrange("t (h w) -> h t w", h=128))
            cs_t = small("cs_t", [128, 32])
            nc.vector.reduce_sum(cs_t, xh.rearrange("h (t w) -> h t w", t=32),
                                 axis=X)
            xsq = big("bigD", [128, 4096])
            TT(xsq, xh, xh, op=Alu.mult)
            cs2_t = small("cs2_t", [128, 32])
            nc.vector.reduce_sum(cs2_t, xsq.rearrange("h (t w) -> h t w", t=32),
                                 axis=X)
            tot = small("B", [128, 512], pb)
            nc.tensor.matmul(tot[0:1, 0:32], ones128, cs_t,
                             start=True, stop=True)
            nc.tensor.matmul(tot[0:1, 32:64], ones128, cs2_t,
                             start=True, stop=True)
            mu_r = small("mu_r", [1, 32])
            nc.scalar.mul(mu_r, tot[0:1, 0:32], 1.0 / 16384.0)
            m2_r = small("m2_r", [1, 32])
            nc.scalar.mul(m2_r, tot[0:1, 32:64], 1.0 / 16384.0)
            mu2_r = small("mu2_r", [1, 32])
            TT(mu2_r, mu_r, mu_r, op=Alu.mult)
            var_r = small("var_r", [1, 32])
            TT(var_r, m2_r, mu2_r, op=Alu.subtract)
            sqv_r = small("sqv_r", [1, 32])
            nc.scalar.activation(sqv_r, var_r, Act.Sqrt, bias=sc32[0:1, 14:15])
            rst_r = small("rst_r", [1, 32])
            nc.vector.reciprocal(rst_r, sqv_r)
            a_r = small("a_r", [1, 32])
            TS(a_r, rst_r, sc32[0:1, 0:1])
            mua_r = small("mua_r", [1, 32])
            TT(mua_r, mu_r, a_r, op=Alu.mult)
            nmua_r = small("nmua_r", [1, 32])
            nc.scalar.mul(nmua_r, mua_r, -1.0)
            c_r = small("c_r", [1, 32])
            nc.vector.tensor_scalar_add(c_r, nmua_r, sc32[0:1, 1:2])
            # transpose [1,32] rows -> [32,1] columns
            atp = small("C", [128, 512], pc)
            nc.tensor.transpose(atp[0:32, 0:1], a_r, iden[0:1, 0:1])
            ctp = small("C", [128, 512], pc)
            nc.tensor.transpose(ctp[0:32, 0:1], c_r, iden[0:1, 0:1])
            a_t = small("a_t", [32, 1])
            CP_(a_t, atp[0:32, 0:1])
            c_t = small("c_t", [32, 1])
            CP_(c_t, ctp[0:32, 0:1])
            a64 = small("a64", [64, 1])
            nc.sync.dma_start(a64[0:32, :], a_t)
            nc.sync.dma_start(a64[32:64, :], a_t)

            # ---------------- forward FFT --------------------------------
            yre = big("bigD", [128, 4096])   # xsq dead
            yim = big("bigE", [128, 4096])
            for (dst, L) in ((yre, ca), (yim, sa)):
                for j in range(8):
                    sl = slice(512 * j, 512 * j + 512)
                    ps = small("A", [128, 512], pa)
                    nc.tensor.matmul(ps, L, xh[:, sl], start=True, stop=True)
                    CP_(dst[:, sl], ps)
            ytre = big("bigF", [128, 4096])
            ytim = big("bigG", [128, 4096])
            for (src, dst) in ((yre, ytre), (yim, ytim)):
                for t in range(32):
                    sl = slice(128 * t, 128 * t + 128)
                    ps = small("A", [128, 512], pa)
                    nc.tensor.transpose(ps[:, 0:128], src[:, sl], iden)
                    CP_(dst[:, sl], ps[:, 0:128])
            zre = big("bigD", [65, 4096])   # yre dead
            zim = big("bigE", [65, 4096])   # yim dead
            for j in range(8):
                sl = slice(512 * j, 512 * j + 512)
                ps = small("A", [128, 512], pa)
                nc.tensor.matmul(ps[0:65, :], lc, ytre[:, sl],
                                 start=True, stop=False)
                nc.tensor.matmul(ps[0:65, :], lsp, ytim[:, sl],
                                 start=False, stop=True)
                CP_(zre[:, sl], ps[0:65, :])
                ps2 = small("B", [128, 512], pb)
                nc.tensor.matmul(ps2[0:65, :], lc, ytim[:, sl],
                                 start=True, stop=False)
                nc.tensor.matmul(ps2[0:65, :], lsn, ytre[:, sl],
                                 start=False, stop=True)
                CP_(zim[:, sl], ps2[0:65, :])
            xf = big("bigA", [64, 8320])
            nc.sync.dma_start(
                xf[0:32, :].rearrange("t (c r) -> t c r", c=65),
                zre.rearrange("c (t r) -> t c r", t=32))
            nc.sync.dma_start(
                xf[32:64, :].rearrange("t (c r) -> t c r", c=65),
                zim.rearrange("c (t r) -> t c r", t=32))
            TS(xf, xf, a64)
            TT(xf[0:32, 0:1], xf[0:32, 0:1], c_t, op=Alu.add)
            if debug:
                nc.sync.dma_start(DXF[:, :], xf)
            if stage <= 1:
                raise StopStage()

            def xcol(plane, cix):
                base = 32 * plane
                return xf[base:base + 32, 128 * cix:128 * cix + 128]

            # ---------------- scores -------------------------------------
            # full-column transposes: xcolT [128 r, 33 c * 32 t] per plane
            xcolT_re = small("xcolT_re", [128, 33 * 32])
            xcolT_im = small("xcolT_im", [128, 33 * 32])
            for cix in range(33):
                for plane, dst in ((0, xcolT_re), (1, xcolT_im)):
                    src = xcol(plane, cix)
                    ps = small("A", [128, 512], pa)
                    nc.tensor.transpose(
                        ps[:, 0:32], src,
                        idens[32 * plane:32 * plane + 32, :])
                    CP_(dst[:, 32 * cix:32 * cix + 32], ps[:, 0:32])
            grp = small("acc", [32, 512], pacc)
            scp = small("acc2", [32, 512], pacc)
            for cix in range(33):
                xr = xcolT_re[:, 32 * cix:32 * cix + 32]
                xi = xcolT_im[:, 32 * cix:32 * cix + 32]
                xr_s = xr[:, :, None].to_broadcast((128, 32, NS))
                xi_s = xi[:, :, None].to_broadcast((128, 32, NS))
                xr_t = xr[:, None, 0:NS].to_broadcast((128, 32, NS))
                xi_t = xi[:, None, 0:NS].to_broadcast((128, 32, NS))
                pre_c = small("pre_c", [128, 256], w2p)
                pim_c = small("pim_c", [128, 256], w2p)
                tmpa = small("tmpa", [128, 256], w2p)
                pr = pre_c.rearrange("p (s t) -> p s t", s=32)
                pi = pim_c.rearrange("p (s t) -> p s t", s=32)
                ta = tmpa.rearrange("p (s t) -> p s t", s=32)
                TT(pr, xr_s, xr_t, op=Alu.mult)
                TT(ta, xi_s, xi_t, op=Alu.mult)
                TT(pr, pr, ta, op=Alu.add)
                TT(pi, xr_s, xi_t, op=Alu.mult)
                TT(ta, xi_s, xr_t, op=Alu.mult)
                TT(pi, pi, ta, op=Alu.subtract)
                nc.tensor.matmul(grp[0:1, 0:256], wrv[:, cix:cix + 1],
                                 pre_c, start=(cix == 0), stop=(cix == 32))
                nc.tensor.matmul(scp[:, 0:256], w2re[:, 32 * cix:32 * cix + 32],
                                 pre_c, start=(cix == 0), stop=False)
                nc.tensor.matmul(scp[:, 0:256],
                                 w2imn[:, 32 * cix:32 * cix + 32],
                                 pim_c, start=False, stop=(cix == 32))
            gr_sb = small("gr_sb", [1, 256])
            CP_(gr_sb, grp[0:1, 0:256])
            scq = small("A", [128, 512], pa)
            nc.tensor.matmul(scq[0:32, 0:256], sqsk, gr_sb,
                             start=True, stop=True)
            sc_sb0 = small("sc_sb0", [32, 256])
            CP_(sc_sb0, scp[:, 0:256])
            sc_sb = small("sc_sb", [32, 256])
            TT(sc_sb, sc_sb0, scq[0:32, 0:256], op=Alu.add)
            if debug:
                nc.sync.dma_start(DSC[:, :], sc_sb)
            if stage <= 2:
                raise StopStage()

            # ---------------- softmax over s ------------------------------
            mx = small("mx", [32, NS])
            nc.vector.reduce_max(
                mx, sc_sb.rearrange("h (s t) -> h t s", s=32), axis=X)
            esub = small("esub", [32, 256])
            TT(esub.rearrange("h (s t) -> h s t", s=32),
               sc_sb.rearrange("h (s t) -> h s t", s=32),
               mx[:, None, :].to_broadcast((32, 32, NS)), op=Alu.subtract)
            ex = small("ex", [32, 256])
            nc.scalar.activation(ex, esub, Act.Exp)
            sm = small("sm", [32, NS])
            nc.vector.reduce_sum(
                sm, ex.rearrange("h (s t) -> h t s", s=32), axis=X)
            rcp = small("rcp", [32, NS])
            nc.vector.reciprocal(rcp, sm)
            attn = small("attn", [32, 256])
            TT(attn.rearrange("h (s t) -> h s t", s=32),
               ex.rearrange("h (s t) -> h s t", s=32),
               rcp[:, None, :].to_broadcast((32, 32, NS)), op=Alu.mult)
            if debug:
                nc.sync.dma_start(DAT[:, :], attn)
            if stage <= 3:
                raise StopStage()

            # ---------------- attnT + CcombT ------------------------------
            ccp = small("B", [128, 512], pb)
            nc.tensor.matmul(ccp[0:1, 0:256], wpssv, attn,
                             start=True, stop=True)
            cc_sb = small("cc_sb", [1, 256])
            CP_(cc_sb, ccp[0:1, 0:256])
            ccT = small("ccT", [64, NS])
            nc.sync.dma_start(ccT[0:32, :],
                              cc_sb.rearrange("o (s t) -> s (o t)", s=32))
            nc.sync.dma_start(ccT[32:64, :],
                              cc_sb.rearrange("o (s t) -> s (o t)", s=32))
            atp1 = small("A", [128, 512], pa)
            nc.tensor.transpose(atp1[:, 0:32], attn[:, 0:128],
                                iden[0:32, 0:32])
            atp1s = small("atp1s", [128, 32])
            CP_(atp1s, atp1[:, 0:32])
            atp2 = small("B", [128, 512], pb)
            nc.tensor.transpose(atp2[:, 0:32], attn[:, 128:256],
                                iden[0:32, 0:32])
            atp2s = small("atp2s", [128, 32])
            CP_(atp2s, atp2[:, 0:32])
            attnT = small("attnT", [64, 256])  # [s, (t, h)] x2 halves
            for base in (0, 32):
                nc.sync.dma_start(
                    attnT[base:base + 16, :].rearrange(
                        "s (t h) -> s t h", t=NS),
                    atp1s.rearrange("(s t) h -> s t h", s=16))
                nc.sync.dma_start(
                    attnT[base + 16:base + 32, :].rearrange(
                        "s (t h) -> s t h", t=NS),
                    atp2s.rearrange("(s t) h -> s t h", s=16))

            # ---------------- AV per-column loop --------------------------
            zp_re = big("bigD", [128, 520])   # zre dead
            zp_im = big("bigE", [128, 520])   # zim dead
            for cix in range(65):
                lre = xcol(0, cix)
                lim = xcol(1, cix)
                cxp = small("C", [128, 512], pc)
                nc.tensor.matmul(cxp[:, 0:NS], lre, ccT[0:32, :],
                                 start=True, stop=True)
                nc.tensor.matmul(cxp[:, NS:2 * NS], lim, ccT[32:64, :],
                                 start=True, stop=True)
                if cix >= 32 or cix >= avlim:
                    CP_(zp_re[:, NS * cix:NS * cix + NS], cxp[:, 0:NS])
                    CP_(zp_im[:, NS * cix:NS * cix + NS], cxp[:, NS:2 * NS])
                    continue
                apr = small("A", [128, 512], pa)
                api = small("A", [128, 512], pa)
                nc.tensor.matmul(apr[:, 0:256], lre, attnT[0:32, :],
                                 start=True, stop=True)
                nc.tensor.matmul(api[:, 0:256], lim, attnT[32:64, :],
                                 start=True, stop=True)
                wvr = wvre[:, 32 * cix:32 * cix + 32][:, None, :].to_broadcast(
                    (128, NS, 32))
                wvi = wvim[:, 32 * cix:32 * cix + 32][:, None, :].to_broadcast(
                    (128, NS, 32))
                wpr = wp2re[:, 32 * cix:32 * cix + 32][:, None, :].to_broadcast(
                    (128, NS, 32))
                wpi = wp2im[:, 32 * cix:32 * cix + 32][:, None, :].to_broadcast(
                    (128, NS, 32))
                aps_r = small("aps_r", [128, 256], w2p)
                aps_i = small("aps_i", [128, 256], w2p)
                CP_(aps_r, apr[:, 0:256])
                CP_(aps_i, api[:, 0:256])
                o64r = small("o64r", [128, 256], w2p)
                o64i = small("o64i", [128, 256], w2p)
                wpo_r = small("wpo_r", [128, 256], w2p)
                wpo_i = small("wpo_i", [128, 256], w2p)
                tv1 = small("tv1", [128, 256], w2p)
                a3r = aps_r.rearrange("p (t h) -> p t h", t=NS)
                a3i = aps_i.rearrange("p (t h) -> p t h", t=NS)
                v1 = tv1.rearrange("p (t h) -> p t h", t=NS)
                o3r = o64r.rearrange("p (t h) -> p t h", t=NS)
                o3i = o64i.rearrange("p (t h) -> p t h", t=NS)
                TT(o3r, a3r, wvr, op=Alu.mult)
                TT(v1, a3i, wvi, op=Alu.mult)
                TT(o3r, o3r, v1, op=Alu.subtract)
                TT(o3i, a3i, wvr, op=Alu.mult)
                TT(v1, a3r, wvi, op=Alu.mult)
                TT(o3i, o3i, v1, op=Alu.add)
                w3r = wpo_r.rearrange("p (t h) -> p t h", t=NS)
                w3i = wpo_i.rearrange("p (t h) -> p t h", t=NS)
                TT(w3r, o3r, wpr, op=Alu.mult)
                TT(v1, o3i, wpi, op=Alu.mult)
                TT(w3r, w3r, v1, op=Alu.subtract)
                TT(w3i, o3i, wpr, op=Alu.mult)
                TT(v1, o3r, wpi, op=Alu.mult)
                TT(w3i, w3i, v1, op=Alu.add)
                nc.vector.reduce_sum(
                    zp_re[:, NS * cix:NS * cix + NS],
                    wpo_r.rearrange("p (t h) -> p t h", t=NS), axis=X)
                nc.vector.reduce_sum(
                    zp_im[:, NS * cix:NS * cix + NS],
                    wpo_i.rearrange("p (t h) -> p t h", t=NS), axis=X)
                CP_(zp_re[32:64, NS * cix:NS * cix + NS], cxp[32:64, 0:NS])
                CP_(zp_re[64:96, NS * cix:NS * cix + NS], cxp[64:96, 0:NS])
                CP_(zp_im[32:64, NS * cix:NS * cix + NS],
                    cxp[32:64, NS:2 * NS])
                CP_(zp_im[64:96, NS * cix:NS * cix + NS],
                    cxp[64:96, NS:2 * NS])
            if debug:
                nc.sync.dma_start(DZPR[:, :], zp_re)
                nc.sync.dma_start(DZPI[:, :], zp_im)
            if stage <= 4:
                raise StopStage()

            # ---------------- irfft2 projd --------------------------------
            y2re = big("bigC", [128, 520])   # xh dead
            y2im = big("bigF", [128, 520])   # pre dead
            y2c = big("bigG", [128, 520])    # pim dead
            for j in range(9):
                n = 64 if j < 8 else 8
                sl = slice(64 * j, 64 * j + n)
                ps = small("A", [128, 512], pa)
                nc.tensor.matmul(ps[:, 0:n], car, zp_re[:, sl],
                                 start=True, stop=True)
                CP_(y2re[:, sl], ps[:, 0:n])
                ps2 = small("B", [128, 512], pb)
                nc.tensor.matmul(ps2[:, 0:n], sar, zp_re[:, sl],
                                 start=True, stop=False)
                nc.tensor.matmul(ps2[:, 0:n], car, zp_im[:, sl],
                                 start=False, stop=True)
                CP_(y2im[:, sl], ps2[:, 0:n])
            for j in range(9):
                n = 64 if j < 8 else 8
                sl = slice(64 * j, 64 * j + n)
                ps = small("A", [128, 512], pa)
                nc.tensor.matmul(ps[:, 0:n], sar, zp_im[:, sl],
                                 start=True, stop=True)
                CP_(y2c[:, sl], ps[:, 0:n])
            TT(y2re, y2re, y2c, op=Alu.subtract)
            projd = big("bigG", [128, 1024])  # y2c dead
            yreT = small("yreT", [65, 128])
            yimT = small("yimT", [65, 128])
            for t in range(NS):
                pt1 = small("A", [128, 512], pa)
                nc.tensor.transpose(
                    pt1[0:65, 0:128],
                    y2re.rearrange("p (c t) -> p t c", t=NS)[:, t, :], iden)
                CP_(yreT, pt1[0:65, 0:128])
                pt2 = small("B", [128, 512], pb)
                nc.tensor.transpose(
                    pt2[0:65, 0:128],
                    y2im.rearrange("p (c t) -> p t c", t=NS)[:, t, :], iden)
                CP_(yimT, pt2[0:65, 0:128])
                pimg = small("A", [128, 512], pa)
                nc.tensor.matmul(pimg[:, 0:128], yreT, gc_,
                                 start=True, stop=False)
                nc.tensor.matmul(pimg[:, 0:128], yimT, gs_,
                                 start=False, stop=True)
                CP_(projd[:, 128 * t:128 * t + 128], pimg[:, 0:128])
            if debug:
                nc.sync.dma_start(DPJ[:, :], projd)
            if stage <= 5:
                raise StopStage()

            # ---------------- norms + mixer -------------------------------
            xsh = big("bigD", [128, 1024])   # zp_re dead
            nc.sync.dma_start(
                xsh.rearrange("h (t w) -> h t w", t=NS),
                x32[0:NS, :].rearrange("t (h w) -> h t w", h=128))

            def inorm_hw(dst, z, gi, bi):
                cs = small("n_cs", [128, NS])
                nc.vector.reduce_sum(
                    cs, z.rearrange("h (t w) -> h t w", t=NS), axis=X)
                z2 = big("n_z2", [128, 1024])
                TT(z2, z, z, op=Alu.mult)
                cs2 = small("n_cs2", [128, NS])
                nc.vector.reduce_sum(
                    cs2, z2.rearrange("h (t w) -> h t w", t=NS), axis=X)
                tt_ = small("B", [128, 512], pb)
                nc.tensor.matmul(tt_[0:1, 0:NS], ones128, cs,
                                 start=True, stop=True)
                nc.tensor.matmul(tt_[0:1, NS:2 * NS], ones128, cs2,
                                 start=True, stop=True)
                mean = small("n_mean", [1, NS])
                nc.scalar.mul(mean, tt_[0:1, 0:NS], 1.0 / 16384.0)
                mean2 = small("n_mean2", [1, NS])
                nc.scalar.mul(mean2, tt_[0:1, NS:2 * NS], 1.0 / 16384.0)
                mm_ = small("n_mm", [1, NS])
                TT(mm_, mean, mean, op=Alu.mult)
                vr = small("n_vr", [1, NS])
                TT(vr, mean2, mm_, op=Alu.subtract)
                sqv = small("n_sqv", [1, NS])
                nc.scalar.activation(sqv, vr, Act.Sqrt, bias=sc32[0:1, 14:15])
                istd0 = small("n_istd0", [1, NS])
                nc.vector.reciprocal(istd0, sqv)
                istd = small("n_istd", [1, NS])
                TS(istd, istd0, sc32[0:1, gi:gi + 1])
                msh0 = small("n_msh0", [1, NS])
                TT(msh0, mean, istd, op=Alu.mult)
                msh1 = small("n_msh1", [1, NS])
                nc.scalar.mul(msh1, msh0, -1.0)
                sh = small("n_sh", [1, NS])
                nc.vector.tensor_scalar_add(sh, msh1, sc32[0:1, bi:bi + 1])
                pl = small("C", [128, 512], pc)
                nc.tensor.matmul(pl[:, 0:NS], ones1, istd,
                                 start=True, stop=True)
                nc.tensor.matmul(pl[:, NS:2 * NS], ones1, sh,
                                 start=True, stop=True)
                pls = small("n_pls", [128, 2 * NS])
                CP_(pls, pl[:, 0:2 * NS])
                TT(dst.rearrange("h (t w) -> h t w", t=NS),
                   z.rearrange("h (t w) -> h t w", t=NS),
                   pls[:, 0:NS][:, :, None].to_broadcast((128, NS, 128)),
                   op=Alu.mult)
                TT(dst.rearrange("h (t w) -> h t w", t=NS),
                   dst.rearrange("h (t w) -> h t w", t=NS),
                   pls[:, NS:2 * NS][:, :, None].to_broadcast((128, NS, 128)),
                   op=Alu.add)

            zsum = big("bigE", [128, 1024])   # zp_im dead
            TT(zsum, projd, xsh, op=Alu.add)
            att = big("att", [128, 1024])
            inorm_hw(att, zsum, 2, 3)
            an = big("bigG", [128, 1024])     # projd dead
            inorm_hw(an, att, 4, 5)
            if debug:
                nc.sync.dma_start(DAN[:, :], an)
            if stage <= 6:
                raise StopStage()

            def mlayer(dst, z, wmre, wmim, gi, bi, wsi, act):
                myre = big("bigE", [64, 1024])
                myim = big("bigF", [64, 1024])
                for j in range(2):
                    sl = slice(512 * j, 512 * j + 512)
                    ps = small("A", [128, 512], pa)
                    nc.tensor.matmul(ps[0:64, :], cam, z[:, sl],
                                     start=True, stop=True)
                    CP_(myre[:, sl], ps[0:64, :])
                    ps2 = small("B", [128, 512], pb)
                    nc.tensor.matmul(ps2[0:64, :], sam, z[:, sl],
                                     start=True, stop=True)
                    CP_(myim[:, sl], ps2[0:64, :])
                mytre = big("bigD", [128, 512])
                mytim = big("bigA", [128, 512])
                for (src, dst2) in ((myre, mytre), (myim, mytim)):
                    for t in range(NS):
                        ps = small("A", [128, 512], pa)
                        nc.tensor.transpose(
                            ps[:, 0:64],
                            src.rearrange("p (t w) -> p t w", t=NS)[:, t, :],
                            iden[0:64, 0:64])
                        CP_(dst2[:, 64 * t:64 * t + 64], ps[:, 0:64])
                mzre = big("bigE", [32, 512])
                mzim = big("bigF", [32, 512])
                ps = small("A", [128, 512], pa)
                nc.tensor.matmul(ps[0:32, :], lcm, mytre, start=True, stop=False)
                nc.tensor.matmul(ps[0:32, :], lspm, mytim, start=False, stop=True)
                CP_(mzre, ps[0:32, :])
                ps2 = small("B", [128, 512], pb)
                nc.tensor.matmul(ps2[0:32, :], lcm, mytim, start=True, stop=False)
                nc.tensor.matmul(ps2[0:32, :], lsnm, mytre, start=False, stop=True)
                CP_(mzim, ps2[0:32, :])
                ofr = big("bigD", [32, 512])
                ofi = big("bigA", [32, 512])
                tvm = small("tvm", [32, 512])
                wr = wmre[:, None, :].to_broadcast((32, NS, 64))
                wi = wmim[:, None, :].to_broadcast((32, NS, 64))
                z3r = mzre.rearrange("c (t r) -> c t r", t=NS)
                z3i = mzim.rearrange("c (t r) -> c t r", t=NS)
                o3r = ofr.rearrange("c (t r) -> c t r", t=NS)
                o3i = ofi.rearrange("c (t r) -> c t r", t=NS)
                tv3 = tvm.rearrange("c (t r) -> c t r", t=NS)
                TT(o3r, z3r, wr, op=Alu.mult)
                TT(tv3, z3i, wi, op=Alu.mult)
                TT(o3r, o3r, tv3, op=Alu.subtract)
                TT(o3i, z3i, wr, op=Alu.mult)
                TT(tv3, z3r, wi, op=Alu.mult)
                TT(o3i, o3i, tv3, op=Alu.add)
                mare = big("bigE", [128, 512])
                maim = big("bigF", [128, 512])
                ps = small("A", [128, 512], pa)
                nc.tensor.matmul(ps, gcm, ofr, start=True, stop=False)
                nc.tensor.matmul(ps, gsmn, ofi, start=False, stop=True)
                CP_(mare, ps)
                ps2 = small("B", [128, 512], pb)
                nc.tensor.matmul(ps2, gsm, ofr, start=True, stop=False)
                nc.tensor.matmul(ps2, gcm, ofi, start=False, stop=True)
                CP_(maim, ps2)
                matre = big("bigD", [64, 1024])
                matim = big("bigA", [64, 1024])
                for (src, dst2) in ((mare, matre), (maim, matim)):
                    for t in range(NS):
                        ps = small("A", [128, 512], pa)
                        nc.tensor.transpose(
                            ps[0:64, 0:128],
                            src.rearrange("p (t r) -> p t r", t=NS)[:, t, :],
                            iden)
                        CP_(dst2[:, 128 * t:128 * t + 128], ps[0:64, 0:128])
                u = big("bigE", [128, 1024])
                uc = big("bigF", [128, 1024])
                for j in range(2):
                    sl = slice(512 * j, 512 * j + 512)
                    ps = small("A", [128, 512], pa)
                    nc.tensor.matmul(ps, carm, matre[:, sl],
                                     start=True, stop=True)
                    CP_(u[:, sl], ps)
                    ps2 = small("B", [128, 512], pb)
                    nc.tensor.matmul(ps2, sarm, matim[:, sl],
                                     start=True, stop=True)
                    CP_(uc[:, sl], ps2)
                TT(u, u, uc, op=Alu.subtract)
                un = big("bigD", [128, 1024])
                inorm_hw(un, u, gi, bi)
                wz = big("bigA", [128, 1024])
                TS(wz, z, sc128[:, wsi:wsi + 1])
                TT(un, un, wz, op=Alu.add)
                if act:
                    nc.scalar.activation(dst, un, Act.Gelu)
                else:
                    CP_(dst, un)

            m0 = big("bigC", [128, 1024])    # y2re dead
            mlayer(m0, an, wm0re, wm0im, 6, 7, 12, True)
            m1 = big("bigG", [128, 1024])    # an dead
            mlayer(m1, m0, wm1re, wm1im, 8, 9, 13, False)
            mn = big("bigE", [128, 1024])    # u dead
            inorm_hw(mn, m1, 10, 11)
            outt = big("bigD", [128, 1024])  # un dead
            TT(outt, mn, att, op=Alu.add)
            nc.sync.dma_start(OUT[:, :], outt)
    nc.compile()
    return nc


def host_inputs(x32_rot, C, W):
    sc = np.zeros(16, np.float32)
    sc[:14] = W["SCAL"]
    sc[14] = EPS
    return {
        "x32": np.ascontiguousarray(x32_rot),
        "CA": C["CA"], "SA": C["SA"], "LC": C["LC"], "LSP": C["LSP"],
        "LSN": C["LSN"], "CAR": C["CAR"], "SAR": C["SAR"],
        "GC": C["GC"], "GS": C["GS"], "CAM": C["CAM"], "SAM": C["SAM"],
        "LCM": C["LCM"], "LSPM": C["LSPM"], "LSNM": C["LSNM"],
        "GCM": C["GCM"], "GSM": C["GSM"], "GSMN": C["GSMN"],
        "CARM": C["CARM"], "SARM": C["SARM"], "IDEN": C["IDEN"],
        "IDENS": np.tile(np.eye(32, dtype=np.float32), (2, 1)),
        "W2RE": W["W2RE"], "W2IMN": W["W2IMN"], "WRV": W["WRV"],
        "SQSK": W["SQSK"],
        "WVRE": np.ascontiguousarray(W["WVRE"].reshape(128, -1)),
        "WVIM": np.ascontiguousarray(W["WVIM"].reshape(128, -1)),
        "WP2RE": np.ascontiguousarray(W["WP2RE"].reshape(128, -1)),
        "WP2IM": np.ascontiguousarray(W["WP2IM"].reshape(128, -1)),
        "WPSSV": W["WPSSV"],
        "WPSSVB": np.ascontiguousarray(
            np.broadcast_to(W["WPSSV"][:, 0], (128, 32))),
        "WM0RE": W["WM0RE"], "WM0IM": W["WM0IM"],
        "WM1RE": W["WM1RE"], "WM1IM": W["WM1IM"],
        "SC32": np.ascontiguousarray(np.broadcast_to(sc, (32, 16))),
        "SC128": np.ascontiguousarray(np.broadcast_to(sc, (128, 16))),
        "ONES128": np.ones((128, 1), np.float32),
        "ONES1": np.ones((1, 128), np.float32),
    }


def kernel(x, wK, wKs, bKs, wQ, wQs, bQs, wV, wVs, bVs, wP, wPs, bPs,
           wM0, wM0s, bM0s, wM1, wM1s, bM1s, norm_g, norm_b):
    global _NC, _CONSTS, LAST_EXEC_NS
    import time

    import concourse.bass_utils as bass_utils

    x = np.asarray(x, dtype=np.float32)
    wts = tuple(np.asarray(a, dtype=np.float32) for a in
                (wK, wKs, wQ, wQs, wV, wVs, wP, wPs,
                 wM0, wM0s, wM1, wM1s, norm_g, norm_b))
    if _CONSTS is None:
        _CONSTS = dft_constants()
    C = _CONSTS
    W = weight_constants(wts)
    if _NC is None:
        _NC = build_nc()

    in_maps = []
    for core in range(8):
        b, sh = divmod(core, 4)
        x32 = np.roll(x[b].reshape(T, -1), -NS * sh, axis=0)
        in_maps.append(host_inputs(np.ascontiguousarray(x32), C, W))
    t0 = time.time()
    res = bass_utils.run_bass_kernel_spmd(_NC, in_maps, core_ids=list(range(8)))
    t1 = time.time()
    LAST_EXEC_NS = (res.exec_time_ns if res.exec_time_ns
                    else int((t1 - t0) * 1e9))
    out = np.empty((B, T, HH, WW), np.float32)
    for core in range(8):
        b, sh = divmod(core, 4)
        o = np.asarray(res.results[core]["OUT"])  # [128 h, (8 t, 128 w)]
        out[b, NS * sh:NS * sh + NS] = o.reshape(
            HH, NS, WW).transpose(1, 0, 2)
    return out
